# revision 21
# baseline (speedup 1.0000x reference)
"""BoxConv2d Trainium2 kernel.

Math: the reference (integral image + bilinear interpolation of fractional
box corners) is algebraically identical to, for each (c, f):

    out[b, c*F+f] = A_cf @ X[b, c] @ B_cf^T

with closed-form interpolation-x-cumsum matrices

    A_cf[h, i] = clip(u1(c,f,h) - i, 0, 1) - clip(u0(c,f,h) - i, 0, 1)
    B_cf[w', j] = clip(v1(c,f,w') - j, 0, 1) - clip(v0(c,f,w') - j, 0, 1)

where u0 = clip(h + x_min*H, 0, H), u1 = clip(h + x_max*H + 1, 0, H) etc.
The tiny A/B matrices are built on host from the box parameters; all
per-sample compute runs on device as dense matmuls on the PE.

Precision: everything runs in single-pass bf16 (inputs, the stage-1
intermediate Z, and the stored output), with fp32 PSUM accumulation
inside each matmul. Measured end-to-end error vs the fp32 reference is
~5e-3 of the output scale (tolerance is 2e-2). The fp32 output array is
reconstructed on host by upcasting, which also halves the HBM store
traffic (the dominant cost at this arithmetic intensity).

Stage order is col-interp first (stationary = X^T, shared over all 8
filters), then row-interp (stationary = A^T, shared over all 8 batch
samples) — this keeps every matmul's moving operand at N=512.

Output is stored in a kernel-private DRAM layout [c, f/2, h, (f%2, b, w)]
so every partition line writes 4 KiB contiguously (large DMA
descriptors ~ full HBM rate); the host-side gather permutes back to
[B, C*F, H, W], which is off the device critical path.

Sharding: channel-parallel — core k handles c in [4k, 4k+4) for all b, f.
"""

import os

import numpy as np

import concourse.bacc as bacc
import concourse.mybir as mybir
import concourse.tile as tile
from concourse import bass_utils

B, C, F, H, W = 8, 32, 8, 128, 128
NCORES = 8
CPC = C // NCORES  # channels per core = 4
BH, FW, FH, BW = B * H, F * W, F * H, B * W  # all 1024
FP = mybir.dt.float32
BF = mybir.dt.bfloat16

_cache = {}


def _build_program():
    if "nc" in _cache:
        return _cache["nc"]

    nc = bacc.Bacc("TRN2", target_bir_lowering=False, debug=False)

    # Fused per-channel input: columns [0:1024) = X^T as [j, (b,i)],
    # [1024:2048) = B^T as [j, (f,w')], [2048:3072) = A^T as [i, (f,h)].
    # One 6 KiB/partition load per channel.
    xba_d = nc.dram_tensor("xba", [CPC, 128, BH + FW + FH], BF,
                           kind="ExternalInput").ap()
    # Private store layout: per (c, f-pair) a [H, 2*B*W] block, 4 KiB
    # contiguous per partition line.
    out_d = nc.dram_tensor("out", [CPC, F // 2, H, 2 * BW], BF,
                           kind="ExternalOutput").ap()

    # Copy-engine schedule: strict alternation keeps both engines draining
    # PSUM in parallel; ACT (measured ~1117 ns/copy) gets one extra vs DVE
    # (~1218 ns/copy) to balance total busy time (33/31). The extra ACT
    # copy goes mid-stream (t=31) where the 4-deep PSUM pool absorbs the
    # hiccup, keeping both the first and last copies one per engine.
    def use_act(t):
        return t % 2 == 0 or t == 31

    with tile.TileContext(nc) as tc:
        with (
            tc.tile_pool(name="wp", bufs=4) as wp,
            # zh: one slot per channel (no write-after-read waits at all);
            # o_t: 6 slots so a pair's copy never waits on an older store
            tc.tile_pool(name="zp", bufs=4) as zp,
            tc.tile_pool(name="op", bufs=6) as op,
            # one shared 4-slot PSUM pool (4 x 2 banks = all 8 banks): the
            # PE can run up to 4 matmul groups ahead of the copy engines,
            # so ACT/DVE copies pack back-to-back (they are the bottleneck)
            tc.tile_pool(name="psp", bufs=4, space="PSUM") as psp,
        ):
            state = {}
            copy_idx = [0]

            def copy_eng():
                t = copy_idx[0]
                copy_idx[0] += 1
                return nc.scalar.copy if use_act(t) else nc.vector.tensor_copy

            def emit_load(c):
                xba_t = wp.tile([128, BH + FW + FH], BF, tag="xba",
                                name=f"xba_{c}")
                if c == 0:
                    # DMA completion is per-instruction and Sync serializes
                    # issues at ~650 ns each, so: keep the first chunks
                    # minimal (b0/b1 stationaries + B^T halves) and issue
                    # the B^T chunks from the Scalar engine's parallel
                    # HWDGE ring (idle until its first PSUM copy) so both
                    # critical chunks are in flight immediately.
                    nc.sync.dma_start(xba_t[:, 0:256], xba_d[c][:, 0:256])
                    nc.scalar.dma_start(xba_t[:, BH : BH + 512],
                                        xba_d[c][:, BH : BH + 512])
                    nc.scalar.dma_start(xba_t[:, BH + 512 : BH + FW],
                                        xba_d[c][:, BH + 512 : BH + FW])
                    nc.sync.dma_start(xba_t[:, 256:BH], xba_d[c][:, 256:BH])
                    nc.sync.dma_start(xba_t[:, BH + FW :],
                                      xba_d[c][:, BH + FW :])
                else:
                    nc.sync.dma_start(xba_t, xba_d[c])
                zh_t = zp.tile([H, B * FW], BF, tag="zh", name=f"zh_{c}")
                state[c] = (xba_t, zh_t)

            def emit_warmup(n):
                # PE warm-up dummies. Measured to be HARMFUL here: the copy
                # engines (the bottleneck) run at full speed regardless of
                # the PE HAM state, and even cold matmul pairs (~960 ns)
                # outpace one copy (~1117 ns) -- dummies only delay the
                # first real results. Kept for A/B experiments; default 0.
                if n <= 0:
                    return
                wsc = wp.tile([128, 512], BF, tag="wsc", name="wsc", bufs=1)
                nc.gpsimd.memset(wsc, 0.0)
                for t in range(n):
                    dpz = psp.tile([H, FW], FP, tag="ps", name=f"warm_{t}")
                    nc.tensor.matmul(dpz[:, 0:512], wsc[:, 0:128], wsc,
                                     start=True, stop=True)

            def emit_s1(c, b):
                # Z_c[i, (b, f, w')] = sum_j X[b,c][i, j] * B[c,f][w', j]
                xba_t, zh_t = state[c]
                pz = psp.tile([H, FW], FP, tag="ps", name=f"pz_{c}_{b}")
                st = xba_t[:, b * H : (b + 1) * H]
                nc.tensor.matmul(pz[:, 0:512], st,
                                 xba_t[:, BH : BH + 512],
                                 start=True, stop=True)
                nc.tensor.matmul(pz[:, 512:1024], st,
                                 xba_t[:, BH + 512 : BH + 1024],
                                 start=True, stop=True)
                copy_eng()(zh_t[:, b * FW : (b + 1) * FW], pz)  # cast to bf16

            def emit_s2(c, f, tail=False):
                # out[b, c*F+f][h, w] = sum_i A[c,f][h, i] * Z_c[i, (b, w)]
                xba_t, zh_t = state[c]
                zh_v = zh_t.rearrange("i (b f w) -> i b f w", b=B, f=F)
                po = psp.tile([H, BW], FP, tag="ps", name=f"po_{c}_{f}")
                st = xba_t[:, BH + FW + f * H : BH + FW + (f + 1) * H]
                nc.tensor.matmul(po[:, 0:512], st, zh_v[:, 0:4, f],
                                 start=True, stop=True)
                nc.tensor.matmul(po[:, 512:1024], st, zh_v[:, 4:8, f],
                                 start=True, stop=True)
                g, fp = f // 2, f % 2
                if fp == 0:
                    state[(c, "o")] = op.tile([H, 2 * BW], BF, tag="o",
                                              name=f"o_{c}_{g}")
                o_t = state[(c, "o")]
                copy_eng()(o_t[:, fp * BW : (fp + 1) * BW], po)  # cast
                if tail:
                    # split the final store so its first half overlaps the
                    # last copy -> shorter drain after compute ends
                    nc.sync.dma_start(out_d[c, g][:, fp * BW : (fp + 1) * BW],
                                      o_t[:, fp * BW : (fp + 1) * BW])
                elif fp == 1:
                    nc.sync.dma_start(out_d[c, g], o_t)

            # Software pipeline: s1 of channel c interleaves with s2 of
            # channel c-1 so the PE always has an alternative matmul group
            # while PSUM banks drain. All loads are issued up front. Half
            # of the second-to-last channel's s2 groups are deferred into
            # the final phase so the copy engines stay busy while the last
            # channel's s1 results (zh) finalize.
            for c in range(CPC):
                emit_load(c)
            emit_warmup(int(os.environ.get('BOXK_WARMUP', '0')))
            for b in range(B):
                emit_s1(0, b)
            for c in range(1, CPC - 1):
                for g in range(B):
                    emit_s1(c, g)
                    emit_s2(c - 1, g)
            for g in range(B):
                emit_s1(CPC - 1, g)
                if g % 2 == 0:
                    emit_s2(CPC - 2, g // 2)
            for f in range(4):
                emit_s2(CPC - 2, 4 + f)
                emit_s2(CPC - 1, f, tail=True)
            for f in range(4, B):
                emit_s2(CPC - 1, f, tail=True)

    nc.compile()
    _cache["nc"] = nc
    return nc


def _host_mats(x_min, x_max, y_min, y_max, max_h, max_w):
    dt = np.float32
    xm = np.asarray(x_min, dt) * dt(max_h)
    xM = np.asarray(x_max, dt) * dt(max_h)
    ym = np.asarray(y_min, dt) * dt(max_w)
    yM = np.asarray(y_max, dt) * dt(max_w)
    h = np.arange(H, dtype=dt)
    w = np.arange(W, dtype=dt)
    u0 = np.clip(h[None, None, :] + xm[:, :, None], 0.0, dt(max_h))
    u1 = np.clip(h[None, None, :] + xM[:, :, None] + dt(1.0), 0.0, dt(max_h))
    v0 = np.clip(w[None, None, :] + ym[:, :, None], 0.0, dt(max_w))
    v1 = np.clip(w[None, None, :] + yM[:, :, None] + dt(1.0), 0.0, dt(max_w))
    i = np.arange(H, dtype=dt)
    A = np.clip(u1[..., None] - i, 0.0, 1.0) - np.clip(u0[..., None] - i, 0.0, 1.0)
    j = np.arange(W, dtype=dt)
    Bm = np.clip(v1[..., None] - j, 0.0, 1.0) - np.clip(v0[..., None] - j, 0.0, 1.0)
    # At[c, i, f, h] = A[c, f, h, i];  Bt[c, j, f, w'] = B[c, f, w', j]
    At = np.ascontiguousarray(np.transpose(A, (0, 3, 1, 2)), dtype=dt)
    Bt = np.ascontiguousarray(np.transpose(Bm, (0, 3, 1, 2)), dtype=dt)
    return At.reshape(C, H, FH), Bt.reshape(C, W, FW)


def _in_maps(input, x_min, x_max, y_min, y_max, max_input_h, max_input_w):
    import ml_dtypes

    x = np.asarray(input, np.float32)
    At, Bt = _host_mats(x_min, x_max, y_min, y_max, int(max_input_h),
                        int(max_input_w))
    # xt[c, j, (b, i)] = x[b, c, i, j]
    xt = np.ascontiguousarray(np.transpose(x, (1, 3, 0, 2))).reshape(C, W, BH)
    xba = np.concatenate([xt, Bt, At], axis=2).astype(ml_dtypes.bfloat16)
    return [{"xba": np.ascontiguousarray(xba[k * CPC : (k + 1) * CPC])}
            for k in range(NCORES)]


def run(inputs, **spmd_kwargs):
    """Build (cached), run on 8 cores, return (full_out, BassKernelResults)."""
    nc = _build_program()
    maps = _in_maps(**inputs)
    res = bass_utils.run_bass_kernel_spmd(
        nc, maps, core_ids=list(range(NCORES)), **spmd_kwargs
    )
    out = np.empty((B, C * F, H, W), np.float32)
    for k in range(NCORES):
        dev = np.asarray(res.results[k]["out"]).reshape(CPC, F // 2, H, 2, B, W)
        out[:, k * CPC * F : (k + 1) * CPC * F] = (
            dev.transpose(4, 0, 1, 3, 2, 5)
            .reshape(B, CPC * F, H, W)
            .astype(np.float32)
        )
    return out, res


def kernel(**inputs) -> np.ndarray:
    out, _ = run(inputs)
    return out
